# revision 7
# baseline (speedup 1.0000x reference)
"""Trainium2 Bass kernel for an 8-head cross-attention block.

Math (per reference):
    Q = video @ Wq[h]           [4096, 64]  per head
    K = text  @ Wk[h]           [1024, 64]
    V = text  @ Wv[h]           [1024, 64]
    att = softmax(Q @ K^T)      [4096, 1024]   (no scaling)
    y_h = att @ V               [4096, 64]
    out = concat_h(y_h) @ Wout + pos_enc(4096, 512)

Sharding: 4 head-groups x 2 query-groups over 8 cores. Core c owns heads
(2*(c//2), 2*(c//2)+1) and queries [(c%2)*2048, (c%2+1)*2048). Each core
emits a full-width [2048, 512] partial of the output projection; the host
sums the 4 head-group partials per query half and adds the positional
encoding.

Two heads per core makes every non-attention matmul full-width on the PE:
the head pair occupies PE columns 0-63/64-127 in the Q/K projections
(stationary [Wq_h0|Wq_h1]) and PE rows 0-63/64-127 in the output
projection (K=128 contraction over both heads' y^T at once). PE cost is
free-dim passes only, so the binding per-core work is the E and att@V
stages (fixed by the math) plus exp on the scalar engine.

Softmax: P = exp(E - 12) via ScalarE on [128, 1024] PSUM tiles; the
denominator comes free as a 65th att@V output row (ones column baked into
the V stationary as [V_h0 | ones | V_h1], host supplies the zero slot).
Per-query normalization happens on the [dh, q] y tiles: DVE reciprocal of
the den row, GpSimd partition-broadcast to 64 rows, one fused DVE
multiply+cast into y_sb. Everything runs fp16 operands with fp32 PSUM.
"""

import numpy as np

from concourse import bacc
import concourse.mybir as mybir
from concourse.tile import TileContext
from concourse.bass_utils import run_bass_kernel_spmd

N, M, D, H, DH = 4096, 1024, 512, 8, 64
P = 128
NL = N // 2          # queries per core (2 query groups)
DC = D // P          # 4 contraction chunks of 128
MT = M // P          # 8 key tiles of 128
NJ = NL // 512       # 4 query chunks of 512 per core
F32 = mybir.dt.float32
FP16 = mybir.dt.float16
FP8 = mybir.dt.float8e4
EXP = mybir.ActivationFunctionType.Exp
EXP_SHIFT = -6.5   # exp(E + shift): max logit ~12.07 -> P <= e^5.6 = 262 < fp8e4m3 max 448
NCORES = 8

_CACHE: dict = {}
TRACE = False          # test harness can flip this before calling kernel()
LAST_RESULT = None     # BassKernelResults of the last run (for profiling)


def _body(tc, nc, vT, tT, wq2, wk2, wv3, wo2, out):
    with tc.tile_pool(name="const", bufs=1) as cp:
        vt_sb = cp.tile([P, DC * NL], FP16, tag="vt")
        tt_sb = cp.tile([P, DC * M], FP16, tag="tt")
        wq_sb = cp.tile([P, DC * P], FP16, tag="wq")
        wk_sb = cp.tile([P, DC * P], FP16, tag="wk")
        wv_sb = cp.tile([P, DC * 160], FP16, tag="wv")
        wo_sb = cp.tile([P, D], FP16, tag="wo")
        qt_sb = cp.tile([P, NL], FP16, tag="qt")     # rows: h0 dh | h1 dh
        kt_sb = cp.tile([P, M], FP16, tag="kt")      # rows: h0 dh | h1 dh
        v_sb = cp.tile([P, MT * 160], FP8, tag="v")   # [V_h0|1|pad|V_h1|1|pad]
        y_sb = cp.tile([P, NL], FP16, tag="y")       # rows: h0 dh | h1 dh
        bias_sb = cp.tile([P, 1], F32, tag="bias")

        for c in range(DC):
            nc.sync.dma_start(out=wk_sb[:, c * P:(c + 1) * P],
                              in_=wk2[c * P:(c + 1) * P, :])
            nc.sync.dma_start(out=tt_sb[:, c * M:(c + 1) * M],
                              in_=tT[c * P:(c + 1) * P, :])
            nc.sync.dma_start(out=wq_sb[:, c * P:(c + 1) * P],
                              in_=wq2[c * P:(c + 1) * P, :])
            nc.sync.dma_start(out=wv_sb[:, c * 160:(c + 1) * 160],
                              in_=wv3[c * P:(c + 1) * P, :])
        for c in range(DC):
            nc.sync.dma_start(out=vt_sb[:, c * NL:(c + 1) * NL],
                              in_=vT[c * P:(c + 1) * P, :])
        nc.sync.dma_start(out=wo_sb[:, :], in_=wo2[:, :])

        v3 = v_sb.rearrange("p (m e) -> p m e", e=160)  # [128, 8, 160]
        nc.vector.memset(bias_sb[:, :], EXP_SHIFT)

        # ---- projections: K^T [128,1024], Q^T [128,2048], V' [128,8,129] ----
        with tc.tile_pool(name="ps_proj", bufs=3, space="PSUM") as pj:
            for mh in range(M // 512):
                ps = pj.tile([P, 512], F32, tag="ps")
                for c in range(DC):
                    nc.tensor.matmul(
                        ps[:, :],
                        wk_sb[:, c * P:(c + 1) * P],
                        tt_sb[:, c * M + mh * 512: c * M + (mh + 1) * 512],
                        start=(c == 0), stop=(c == DC - 1))
                nc.vector.tensor_copy(
                    out=kt_sb[:, mh * 512:(mh + 1) * 512], in_=ps[:, :])
            for j in range(NJ):
                ps = pj.tile([P, 512], F32, tag="ps")
                for c in range(DC):
                    nc.tensor.matmul(
                        ps[:, :],
                        wq_sb[:, c * P:(c + 1) * P],
                        vt_sb[:, c * NL + j * 512: c * NL + (j + 1) * 512],
                        start=(c == 0), stop=(c == DC - 1))
                nc.vector.tensor_copy(
                    out=qt_sb[:, j * 512:(j + 1) * 512], in_=ps[:, :])
            for mt in range(MT):
                ps = pj.tile([P, 160], F32, tag="psv")
                for c in range(DC):
                    nc.tensor.matmul(
                        ps[:, :],
                        tt_sb[:, c * M + mt * P: c * M + (mt + 1) * P],
                        wv_sb[:, c * 160:(c + 1) * 160],
                        start=(c == 0), stop=(c == DC - 1))
                nc.vector.tensor_copy(out=v3[:, mt, :], in_=ps[:, :])
        nc.vector.memset(v3[:, :, DH], 1.0)
        nc.vector.memset(v3[:, :, 144], 1.0)

        # ---- attention: E^T -> exp -> y^T (+den row) -> normalize ----
        out_r = out.rearrange("(g p) d -> p g d", p=P)  # [128, 16, 512]

        def emit_out(j, po_pool, o_pool):
            for nt in range(j * 4, (j + 1) * 4):
                ps = po_pool.tile([P, D], F32, tag="po")
                nc.tensor.matmul(
                    ps[:, :],
                    y_sb[:, nt * P:(nt + 1) * P],
                    wo_sb[:, :],
                    start=True, stop=True)
                ot = o_pool.tile([P, D], FP16, tag="o")
                nc.vector.tensor_copy(out=ot[:, :], in_=ps[:, :])
                nc.sync.dma_start(out=out_r[:, nt, :], in_=ot[:, :])

        with tc.tile_pool(name="ps_e", bufs=2, space="PSUM") as e_pool, \
             tc.tile_pool(name="ps_y", bufs=2, space="PSUM") as y_pool, \
             tc.tile_pool(name="ps_o", bufs=2, space="PSUM") as po_pool, \
             tc.tile_pool(name="p_sb", bufs=6) as p_pool, \
             tc.tile_pool(name="o_sb", bufs=4) as o_pool, \
             tc.tile_pool(name="nrm", bufs=2) as n_pool:
            for j in range(NJ):
                jsl = slice(j * 512, (j + 1) * 512)
                for h in range(2):
                    if j > 0 and h == 1:
                        emit_out(j - 1, po_pool, o_pool)
                    hs = slice(h * DH, (h + 1) * DH)
                    p_tiles = []
                    for tp in range(MT // 2):
                        e_ps = e_pool.tile([P, 1024], F32, tag="e")
                        for i in range(2):
                            mt = 2 * tp + i
                            nc.tensor.matmul(
                                e_ps[:, i * 512:(i + 1) * 512],
                                kt_sb[hs, mt * P:(mt + 1) * P],
                                qt_sb[hs, jsl],
                                start=True, stop=True)
                        pt = p_pool.tile([P, 1024], FP8, tag="p")
                        nc.scalar.activation(pt[:, :], e_ps[:, :], EXP,
                                             bias=bias_sb[:, :])
                        p_tiles.append(pt)
                    y_ps = y_pool.tile([DH + 1, 512], F32, tag="y")
                    vsl = slice(0, DH + 1) if h == 0 else slice(80, 80 + DH + 1)
                    for tp in range(MT // 2):
                        pt2 = p_tiles[tp].rearrange("k (i q) -> k i q", i=2)
                        nc.tensor.matmul(
                            y_ps[:, :],
                            v3[:, 2 * tp:2 * tp + 2, vsl],
                            pt2[:, :, :],
                            start=(tp == 0), stop=(tp == MT // 2 - 1),
                            perf_mode=mybir.MatmulPerfMode.DoubleRow)
                    # normalize: den is always row 64 of y_ps
                    den = n_pool.tile([1, 512], F32, tag="den")
                    rden = n_pool.tile([1, 512], F32, tag="rden")
                    bc = n_pool.tile([DH, 512], F32, tag="bc")
                    nc.vector.tensor_copy(out=den[:, :], in_=y_ps[DH:DH + 1, :])
                    nc.vector.reciprocal_approx_fast(rden[:, :], den[:, :])
                    nc.gpsimd.partition_broadcast(bc[:, :], rden[:, :])
                    nc.vector.tensor_mul(y_sb[hs, jsl], y_ps[0:DH, :], bc[:, :])
            emit_out(NJ - 1, po_pool, o_pool)


def _build():
    nc = bacc.Bacc("TRN2", target_bir_lowering=False, debug=False)
    vT = nc.dram_tensor("vT", [D, NL], FP16, kind="ExternalInput")
    tT = nc.dram_tensor("tT", [D, M], FP16, kind="ExternalInput")
    wq2 = nc.dram_tensor("wq2", [D, P], FP16, kind="ExternalInput")
    wk2 = nc.dram_tensor("wk2", [D, P], FP16, kind="ExternalInput")
    wv3 = nc.dram_tensor("wv3", [D, 160], FP16, kind="ExternalInput")
    wo2 = nc.dram_tensor("wo2", [P, D], FP16, kind="ExternalInput")
    out = nc.dram_tensor("out", [NL, D], FP16, kind="ExternalOutput")
    with TileContext(nc) as tc:
        _body(tc, nc, vT[:, :], tT[:, :], wq2[:, :], wk2[:, :], wv3[:, :],
              wo2[:, :], out[:, :])
    nc.compile()
    return nc


def _pos_encoding():
    # Mirror the reference's jnp ops bit-for-bit (numpy's f32 sin/exp differ
    # by enough ULPs to dominate the error budget at pos/freq ~ 4e3).
    import jax
    import jax.numpy as jnp
    with jax.default_device(jax.devices("cpu")[0]):
        pos = jnp.arange(N, dtype=jnp.float32)
        freq = jnp.exp(
            (jnp.arange(D // 2, dtype=jnp.float32) / D)
            * jnp.log(jnp.float32(10000.0)))
        x = pos[:, None] / freq
        pe = jnp.stack((jnp.sin(x), jnp.cos(x)), axis=-1)
        return np.asarray(pe.reshape(N, D), dtype=np.float32)


def _fp16(a):
    return np.ascontiguousarray(np.asarray(a, dtype=np.float32).astype(np.float16))


def kernel(video_features, text_features, Wq, Wk, Wv, Wout):
    global LAST_RESULT
    if "nc" not in _CACHE:
        _CACHE["nc"] = _build()
        _CACHE["pe"] = _pos_encoding()
    nc = _CACHE["nc"]

    vT = np.asarray(video_features, dtype=np.float32).T
    tT = _fp16(np.asarray(text_features, dtype=np.float32).T)
    Wq = np.asarray(Wq, dtype=np.float32)
    Wk = np.asarray(Wk, dtype=np.float32)
    Wv = np.asarray(Wv, dtype=np.float32)
    Wout = np.asarray(Wout, dtype=np.float32)
    z1 = np.zeros((D, 1), dtype=np.float32)
    z15 = np.zeros((D, 15), dtype=np.float32)

    in_maps = []
    for c in range(NCORES):
        hg, qg = c // 2, c % 2
        h0, h1 = 2 * hg, 2 * hg + 1
        in_maps.append({
            "vT": _fp16(vT[:, qg * NL:(qg + 1) * NL]),
            "tT": tT,
            "wq2": _fp16(np.concatenate([Wq[h0], Wq[h1]], axis=1)),
            "wk2": _fp16(np.concatenate([Wk[h0], Wk[h1]], axis=1)),
            "wv3": _fp16(np.concatenate([Wv[h0], z1, z15, Wv[h1], z1, z15], axis=1)),
            "wo2": _fp16(Wout[h0 * DH:(h1 + 1) * DH, :]),
        })
    res = run_bass_kernel_spmd(nc, in_maps, list(range(NCORES)), trace=TRACE)
    LAST_RESULT = res
    acc = np.zeros((N, D), dtype=np.float32)
    for c in range(NCORES):
        hg, qg = c // 2, c % 2
        acc[qg * NL:(qg + 1) * NL] += res.results[c]["out"].astype(np.float32)
    return (acc + _CACHE["pe"]).astype(np.float32)


# revision 8
# speedup vs baseline: 1.1014x; 1.1014x over previous
"""Trainium2 Bass kernel for an 8-head cross-attention block.

Math (per reference):
    Q = video @ Wq[h]           [4096, 64]  per head
    K = text  @ Wk[h]           [1024, 64]
    V = text  @ Wv[h]           [1024, 64]
    att = softmax(Q @ K^T)      [4096, 1024]   (no scaling)
    y_h = att @ V               [4096, 64]
    out = concat_h(y_h) @ Wout + pos_enc(4096, 512)

Sharding: 4 head-groups x 2 query-groups over 8 cores. Core c owns heads
(2*(c//2), 2*(c//2)+1) and queries [(c%2)*2048, (c%2+1)*2048). Each core
emits a full-width [2048, 512] partial of the output projection; the host
sums the 4 head-group partials per query half and adds the positional
encoding.

Two heads per core makes every non-attention matmul full-width on the PE:
the head pair occupies PE columns 0-63/64-127 in the Q/K projections
(stationary [Wq_h0|Wq_h1]) and PE rows 0-63/64-127 in the output
projection (K=128 contraction over both heads' y^T at once). PE cost is
free-dim passes only, so the binding per-core work is the E and att@V
stages (fixed by the math) plus exp on the scalar engine.

Softmax: P = exp(E - 12) via ScalarE on [128, 1024] PSUM tiles; the
denominator comes free as a 65th att@V output row (ones column baked into
the V stationary as [V_h0 | ones | V_h1], host supplies the zero slot).
Per-query normalization happens on the [dh, q] y tiles: DVE reciprocal of
the den row, GpSimd partition-broadcast to 64 rows, one fused DVE
multiply+cast into y_sb. Everything runs fp16 operands with fp32 PSUM.
"""

import numpy as np

from concourse import bacc
import concourse.mybir as mybir
from concourse.tile import TileContext
from concourse.bass_utils import run_bass_kernel_spmd

N, M, D, H, DH = 4096, 1024, 512, 8, 64
P = 128
NL = N // 2          # queries per core (2 query groups)
DC = D // P          # 4 contraction chunks of 128
MT = M // P          # 8 key tiles of 128
NJ = NL // 512       # 4 query chunks of 512 per core
F32 = mybir.dt.float32
FP16 = mybir.dt.bfloat16
FP8 = mybir.dt.float8e4
EXP = mybir.ActivationFunctionType.Exp
EXP_SHIFT = -6.5   # exp(E + shift): max logit ~12.07 -> P <= e^5.6 = 262 < fp8e4m3 max 448
NCORES = 8

_CACHE: dict = {}
TRACE = False          # test harness can flip this before calling kernel()
LAST_RESULT = None     # BassKernelResults of the last run (for profiling)


def _body(tc, nc, vT, tT, wq2, wk2, wv3, wo2, out):
    with tc.tile_pool(name="const", bufs=1) as cp:
        vt_sb = cp.tile([P, DC * NL], FP16, tag="vt")
        tt_sb = cp.tile([P, DC * M], FP16, tag="tt")
        wq_sb = cp.tile([P, DC * P], FP16, tag="wq")
        wk_sb = cp.tile([P, DC * P], FP16, tag="wk")
        wv_sb = cp.tile([P, DC * 160], FP16, tag="wv")
        wo_sb = cp.tile([P, D], FP16, tag="wo")
        qt_sb = cp.tile([P, NL], FP16, tag="qt")     # rows: h0 dh | h1 dh
        kt_sb = cp.tile([P, M], FP16, tag="kt")      # rows: h0 dh | h1 dh
        v_sb = cp.tile([P, MT * 160], FP16, tag="v")  # [V_h0|1|pad|V_h1|1|pad]
        y_sb = cp.tile([P, NL], FP16, tag="y")       # rows: h0 dh | h1 dh
        bias_sb = cp.tile([P, 1], F32, tag="bias")

        for c in range(DC):
            nc.sync.dma_start(out=wk_sb[:, c * P:(c + 1) * P],
                              in_=wk2[c * P:(c + 1) * P, :])
            nc.sync.dma_start(out=tt_sb[:, c * M:(c + 1) * M],
                              in_=tT[c * P:(c + 1) * P, :])
            nc.sync.dma_start(out=wq_sb[:, c * P:(c + 1) * P],
                              in_=wq2[c * P:(c + 1) * P, :])
            nc.sync.dma_start(out=wv_sb[:, c * 160:(c + 1) * 160],
                              in_=wv3[c * P:(c + 1) * P, :])
        for c in range(DC):
            nc.sync.dma_start(out=vt_sb[:, c * NL:(c + 1) * NL],
                              in_=vT[c * P:(c + 1) * P, :])
        nc.sync.dma_start(out=wo_sb[:, :], in_=wo2[:, :])

        v3 = v_sb.rearrange("p (m e) -> p m e", e=160)  # [128, 8, 160]
        nc.vector.memset(bias_sb[:, :], EXP_SHIFT)

        # ---- projections: K^T [128,1024], Q^T [128,2048], V' [128,8,129] ----
        with tc.tile_pool(name="ps_proj", bufs=3, space="PSUM") as pj:
            for mh in range(M // 512):
                ps = pj.tile([P, 512], F32, tag="ps")
                for c in range(DC):
                    nc.tensor.matmul(
                        ps[:, :],
                        wk_sb[:, c * P:(c + 1) * P],
                        tt_sb[:, c * M + mh * 512: c * M + (mh + 1) * 512],
                        start=(c == 0), stop=(c == DC - 1))
                nc.vector.tensor_copy(
                    out=kt_sb[:, mh * 512:(mh + 1) * 512], in_=ps[:, :])
            for j in range(NJ):
                ps = pj.tile([P, 512], F32, tag="ps")
                for c in range(DC):
                    nc.tensor.matmul(
                        ps[:, :],
                        wq_sb[:, c * P:(c + 1) * P],
                        vt_sb[:, c * NL + j * 512: c * NL + (j + 1) * 512],
                        start=(c == 0), stop=(c == DC - 1))
                nc.vector.tensor_copy(
                    out=qt_sb[:, j * 512:(j + 1) * 512], in_=ps[:, :])
            for mt in range(MT):
                ps = pj.tile([P, 160], F32, tag="psv")
                for c in range(DC):
                    nc.tensor.matmul(
                        ps[:, :],
                        tt_sb[:, c * M + mt * P: c * M + (mt + 1) * P],
                        wv_sb[:, c * 160:(c + 1) * 160],
                        start=(c == 0), stop=(c == DC - 1))
                nc.vector.tensor_copy(out=v3[:, mt, :], in_=ps[:, :])
        nc.vector.memset(v3[:, :, DH], 1.0)
        nc.vector.memset(v3[:, :, 144], 1.0)

        # ---- attention: E^T -> exp -> y^T (+den row) -> normalize ----
        out_r = out.rearrange("(g p) d -> p g d", p=P)  # [128, 16, 512]

        def emit_out(j, po_pool, o_pool):
            for nt in range(j * 4, (j + 1) * 4):
                ps = po_pool.tile([P, D], F32, tag="po")
                nc.tensor.matmul(
                    ps[:, :],
                    y_sb[:, nt * P:(nt + 1) * P],
                    wo_sb[:, :],
                    start=True, stop=True)
                ot = o_pool.tile([P, D], FP16, tag="o")
                nc.vector.tensor_copy(out=ot[:, :], in_=ps[:, :])
                nc.sync.dma_start(out=out_r[:, nt, :], in_=ot[:, :])

        with tc.tile_pool(name="ps_e", bufs=2, space="PSUM") as e_pool, \
             tc.tile_pool(name="ps_y", bufs=2, space="PSUM") as y_pool, \
             tc.tile_pool(name="ps_o", bufs=2, space="PSUM") as po_pool, \
             tc.tile_pool(name="p_sb", bufs=6) as p_pool, \
             tc.tile_pool(name="o_sb", bufs=4) as o_pool, \
             tc.tile_pool(name="nrm", bufs=2) as n_pool:
            for j in range(NJ):
                jsl = slice(j * 512, (j + 1) * 512)
                for h in range(2):
                    if j > 0 and h == 1:
                        emit_out(j - 1, po_pool, o_pool)
                    hs = slice(h * DH, (h + 1) * DH)
                    p_tiles = []
                    for tp in range(MT // 2):
                        e_ps = e_pool.tile([P, 1024], F32, tag="e")
                        for i in range(2):
                            mt = 2 * tp + i
                            nc.tensor.matmul(
                                e_ps[:, i * 512:(i + 1) * 512],
                                kt_sb[hs, mt * P:(mt + 1) * P],
                                qt_sb[hs, jsl],
                                start=True, stop=True)
                        pt = p_pool.tile([P, 1024], FP16, tag="p")
                        nc.scalar.activation(pt[:, :], e_ps[:, :], EXP,
                                             bias=bias_sb[:, :])
                        p_tiles.append(pt)
                    y_ps = y_pool.tile([DH + 1, 512], F32, tag="y")
                    vsl = slice(0, DH + 1) if h == 0 else slice(80, 80 + DH + 1)
                    for mt in range(MT):
                        nc.tensor.matmul(
                            y_ps[:, :],
                            v3[:, mt, vsl],
                            p_tiles[mt // 2][:, (mt % 2) * 512:(mt % 2 + 1) * 512],
                            start=(mt == 0), stop=(mt == MT - 1))
                    # normalize: den is always row 64 of y_ps
                    den = n_pool.tile([1, 512], F32, tag="den")
                    rden = n_pool.tile([1, 512], F32, tag="rden")
                    bc = n_pool.tile([DH, 512], F32, tag="bc")
                    nc.vector.tensor_copy(out=den[:, :], in_=y_ps[DH:DH + 1, :])
                    nc.vector.reciprocal_approx_fast(rden[:, :], den[:, :])
                    nc.gpsimd.partition_broadcast(bc[:, :], rden[:, :])
                    nc.vector.tensor_mul(y_sb[hs, jsl], y_ps[0:DH, :], bc[:, :])
            emit_out(NJ - 1, po_pool, o_pool)


def _build():
    nc = bacc.Bacc("TRN2", target_bir_lowering=False, debug=False)
    vT = nc.dram_tensor("vT", [D, NL], FP16, kind="ExternalInput")
    tT = nc.dram_tensor("tT", [D, M], FP16, kind="ExternalInput")
    wq2 = nc.dram_tensor("wq2", [D, P], FP16, kind="ExternalInput")
    wk2 = nc.dram_tensor("wk2", [D, P], FP16, kind="ExternalInput")
    wv3 = nc.dram_tensor("wv3", [D, 160], FP16, kind="ExternalInput")
    wo2 = nc.dram_tensor("wo2", [P, D], FP16, kind="ExternalInput")
    out = nc.dram_tensor("out", [NL, D], FP16, kind="ExternalOutput")
    with TileContext(nc) as tc:
        _body(tc, nc, vT[:, :], tT[:, :], wq2[:, :], wk2[:, :], wv3[:, :],
              wo2[:, :], out[:, :])
    nc.compile()
    return nc


def _pos_encoding():
    # Mirror the reference's jnp ops bit-for-bit (numpy's f32 sin/exp differ
    # by enough ULPs to dominate the error budget at pos/freq ~ 4e3).
    import jax
    import jax.numpy as jnp
    with jax.default_device(jax.devices("cpu")[0]):
        pos = jnp.arange(N, dtype=jnp.float32)
        freq = jnp.exp(
            (jnp.arange(D // 2, dtype=jnp.float32) / D)
            * jnp.log(jnp.float32(10000.0)))
        x = pos[:, None] / freq
        pe = jnp.stack((jnp.sin(x), jnp.cos(x)), axis=-1)
        return np.asarray(pe.reshape(N, D), dtype=np.float32)


def _fp16(a):
    import ml_dtypes
    return np.ascontiguousarray(
        np.asarray(a, dtype=np.float32).astype(ml_dtypes.bfloat16))


def kernel(video_features, text_features, Wq, Wk, Wv, Wout):
    global LAST_RESULT
    if "nc" not in _CACHE:
        _CACHE["nc"] = _build()
        _CACHE["pe"] = _pos_encoding()
    nc = _CACHE["nc"]

    vT = np.asarray(video_features, dtype=np.float32).T
    tT = _fp16(np.asarray(text_features, dtype=np.float32).T)
    Wq = np.asarray(Wq, dtype=np.float32)
    Wk = np.asarray(Wk, dtype=np.float32)
    Wv = np.asarray(Wv, dtype=np.float32)
    Wout = np.asarray(Wout, dtype=np.float32)
    z1 = np.zeros((D, 1), dtype=np.float32)
    z15 = np.zeros((D, 15), dtype=np.float32)

    in_maps = []
    for c in range(NCORES):
        hg, qg = c // 2, c % 2
        h0, h1 = 2 * hg, 2 * hg + 1
        in_maps.append({
            "vT": _fp16(vT[:, qg * NL:(qg + 1) * NL]),
            "tT": tT,
            "wq2": _fp16(np.concatenate([Wq[h0], Wq[h1]], axis=1)),
            "wk2": _fp16(np.concatenate([Wk[h0], Wk[h1]], axis=1)),
            "wv3": _fp16(np.concatenate([Wv[h0], z1, z15, Wv[h1], z1, z15], axis=1)),
            "wo2": _fp16(Wout[h0 * DH:(h1 + 1) * DH, :]),
        })
    res = run_bass_kernel_spmd(nc, in_maps, list(range(NCORES)), trace=TRACE)
    LAST_RESULT = res
    acc = np.zeros((N, D), dtype=np.float32)
    for c in range(NCORES):
        hg, qg = c // 2, c % 2
        acc[qg * NL:(qg + 1) * NL] += res.results[c]["out"].astype(np.float32)
    return (acc + _CACHE["pe"]).astype(np.float32)
